# revision 1
# baseline (speedup 1.0000x reference)
"""Trainium2 Bass kernel for ActiveSpline (centripetal Catmull-Rom spline sampling).

Full input:  cps [16384, 16, 2] f32
Full output: pts [16384, 1024, 2] f32

Math: for each batch, build the closed-curve auxiliary control points and
centripetal knot *differences* (only knot diffs appear in the Barry-Goldman
pyramid), reduce each segment's 64-point evaluation to a cubic polynomial in
the normalized parameter u in [0,1]:

    pts(u) = g0 + g1 u + g2 u^2 + g3 u^3        (per batch, segment, coord)

The g-coefficients are computed on the Vector engine (batch-major layout:
partition p holds batches 16p..16p+15), then expanded to the 64 sample points
with TensorEngine matmuls against a constant block-Vandermonde matrix:

    out[b, (s,p,d)] = sum_{(s,d,k)} coefT[(s,d,k), b] * W[(s,d,k), (s,p,d)]

The matmul runs as a 3-pass fp16 hi/lo split (C=Chi+Clo, W=Whi+Wlo;
out = Chi*Whi + Chi*Wlo + Clo*Whi, dropping the ~2^-22 Clo*Wlo term) which is
4x faster than fp32 matmuls at ~1e-6 relative error.

Work is pipelined in 4 groups of 4 batch-rows-per-partition so the vector
engine's coefficient pass overlaps the PE expansion, PSUM drains, and the
output DMA.

Sharding: purely batch-parallel, 2048 batches per core on 8 cores.
"""

import numpy as np

import concourse.bass as bass
import concourse.bacc as bacc
import concourse.mybir as mybir
import concourse.tile as tile
from concourse.bass_utils import run_bass_kernel_spmd

F32 = mybir.dt.float32
F16 = mybir.dt.float16
AF = mybir.ActivationFunctionType

N_CORES = 8
B_FULL = 16384
S = 16            # control points / segments (closed curve)
D = 2
P = 64            # samples per segment
BC = B_FULL // N_CORES   # 2048 batches per core
J = 16            # batches per partition (BC = 128 * J)
JG = 4            # j's per pipeline group
NG = J // JG      # 4 groups
EPS = 1e-7
OUTW = S * P * D  # 2048 output floats per batch

# "rowtile": 4 concurrent K=32 matmuls via tile_position; "fullk": K=128
MATMUL_MODE = "rowtile"
# "f16x3": 3-pass fp16 hi/lo split; "f32": plain fp32 matmuls (4 cyc/row)
MM_DTYPE = "f16x3"
# columns of each 2048-col PSUM output drained by the vector engine
# (the rest goes to the scalar engine, which is otherwise idle);
# 576 balances total engine busy (DVE also runs the coefficient pass)
DVE_COPY_COLS = 576  # of 2048
OSG = 2           # j's per output DMA (16KB/partition contiguous runs)
DUAL_DMA = False  # alternate output DMAs between the two HWDGE rings
# Ramp-aware output DMA schedule: (start_j, end_j) per DMA. Small leading
# chunks start the HBM drain as early as possible; 2MB chunks follow for
# descriptor efficiency. Measured 2.2us better than uniform 2MB chunks
# (interleaved above-quantum protocol). None -> uniform OSG-sized chunks.
DMA_SCHED = ((0, 1), (1, 2), (2, 4), (4, 6), (6, 8), (8, 10), (10, 12),
             (12, 14), (14, 16))
INPUT_SPLIT = True  # split the input DMA per coefficient group
# dummy fp16 matmuls issued during the input-DMA/coefficient ramp so the
# PE HAM clock-gate is already released (2.4 GHz) when the real expansion
# matmuls arrive. They accumulate into a scratch PSUM bank nobody reads.
# Measured neutral in steady-state loops, helps the cold single-shot ramp.
WARM_MM = 12


def _build_w() -> np.ndarray:
    """W [128, 2048] f32: block-diagonal expansion matrix.

    Row r = 32*c + rr encodes (s = 4*c + (rr>>3), d = (rr>>2)&1, k = rr&3).
    Col n = 512*c' + nn encodes (s' = 4*c' + nn//128, p = (nn%128)//2, d' = nn&1).
    W[r, n] = (s==s' and d==d') * u_p^k.
    """
    f = np.float32
    u = (np.arange(P, dtype=f) / f(P - 1)).astype(f)
    pow_u = np.stack([np.ones(P, f), u, u * u, (u * u) * u])  # [4, 64]
    w4 = np.zeros((32, 512), f)
    for rr in range(32):
        sl, d, k = rr >> 3, (rr >> 2) & 1, rr & 3
        w4[rr, sl * 128 + np.arange(P) * 2 + d] = pow_u[k]
    w = np.zeros((128, 2048), f)
    for c in range(4):
        w[32 * c:32 * c + 32, 512 * c:512 * (c + 1)] = w4
    return w


def _build_nc(n_reps: int = 1, loop_n: int = 0):
    nc = bacc.Bacc("TRN2", target_bir_lowering=False, debug=False,
                   enable_asserts=False, num_devices=N_CORES)

    f16x3 = MM_DTYPE == "f16x3"
    cps_d = nc.dram_tensor("cps", [BC, S * D], F32, kind="ExternalInput")
    if f16x3:
        whi_d = nc.dram_tensor("whi", [128, OUTW], F16, kind="ExternalInput")
        wlo_d = nc.dram_tensor("wlo", [128, OUTW], F16, kind="ExternalInput")
    else:
        w_d = nc.dram_tensor("wmat", [128, OUTW], F32, kind="ExternalInput")
    id_d = nc.dram_tensor("ident", [128, 128], F32, kind="ExternalInput")
    out_d = nc.dram_tensor("out", [BC, OUTW], F32, kind="ExternalOutput")

    with tile.TileContext(nc) as tc:
        with (
            tc.tile_pool(name="const", bufs=1) as const,
            tc.tile_pool(name="inp", bufs=2) as inp,
            tc.tile_pool(name="work", bufs=2) as work,
            tc.tile_pool(name="lhs", bufs=2 * JG + 2) as lhsp,
            tc.tile_pool(name="osb", bufs=2) as osbp,
            tc.tile_pool(name="pst", bufs=2, space="PSUM") as pst,
            tc.tile_pool(name="pso", bufs=3, space="PSUM") as psop,
        ):
            # ---- constants ----
            if f16x3:
                whi_sb = const.tile([128, OUTW], F16)
                wlo_sb = const.tile([128, OUTW], F16)
                nc.sync.dma_start(whi_sb[:], whi_d.ap())
                nc.sync.dma_start(wlo_sb[:], wlo_d.ap())
                wmats = (whi_sb, wlo_sb)
            else:
                w_sb = const.tile([128, OUTW], F32)
                nc.sync.dma_start(w_sb[:], w_d.ap())
                wmats = (w_sb,)
            i_sb = const.tile([128, 128], F32)
            nc.sync.dma_start(i_sb[:], id_d.ap())

            eps_t = const.tile([128, 1], F32)
            zero_t = const.tile([128, 1], F32)
            nc.vector.memset(eps_t[:], float(EPS))
            nc.vector.memset(zero_t[:], 0.0)
            # dummy Sqrt so the ACT table-set load (~2.7us) happens during
            # the input DMA instead of on phase B's critical chain
            warm_act = const.tile([128, 1], F32)
            nc.scalar.activation(warm_act[:], eps_t[:], AF.Sqrt,
                                 bias=zero_t[:])

            consts = (cps_d, out_d, wmats, i_sb, eps_t, zero_t)
            pools = (inp, work, lhsp, osbp, pst, psop)
            if loop_n:
                with tc.For_i(0, loop_n, 1):
                    _emit_once(nc, tc, pools, consts)
            else:
                for _rep in range(n_reps):
                    _emit_once(nc, tc, pools, consts)

    nc.compile()
    return nc


def _emit_once(nc, tc, pools, consts):
    inp, work, lhsp, osbp, pst, psop = pools
    cps_d, out_d, wmats, i_sb, eps_t, zero_t = consts
    f16x3 = MM_DTYPE == "f16x3"

    # ---- input (all 16 j's at once: partition p holds batches 16p..16p+15) --
    x = inp.tile([128, J * S * D], F32)
    xj = x[:].rearrange("p (j q) -> p j q", j=J)
    cj = cps_d.ap().rearrange("(p j) q -> p j q", j=J)
    ng = J // JG
    if INPUT_SPLIT:
        for g in range(ng):
            nc.sync.dma_start(xj[:, g * JG:(g + 1) * JG, :],
                              cj[:, g * JG:(g + 1) * JG, :])
    else:
        nc.sync.dma_start(xj, cj)
    xall = x[:].rearrange("p (j s d) -> p j s d", j=J, s=S, d=D)
    if WARM_MM and f16x3:
        wt = pst.tile([128, 512], F32, tag="tp", name="warm")
        for i in range(WARM_MM):
            nc.tensor.matmul(wt[:], wmats[0][:, 0:128], wmats[0][:, 0:512],
                             start=(i == 0), stop=(i == WARM_MM - 1))
    sched = (DMA_SCHED if DMA_SCHED is not None
             else tuple((a, a + OSG) for a in range(0, J, OSG)))
    dma_of = {}          # last j of a chunk -> (start, end)
    for (a, b) in sched:
        dma_of[b - 1] = (a, b)
    outj = out_d.ap().rearrange("(p j) q -> p j q", j=J)

    osb = None
    osb_base = 0
    for g in range(ng):
        xv = xall[:, g * JG:(g + 1) * JG, :, :]
        coef = _phase_b(nc, work, xv, eps_t, zero_t)

        # transposes + fp16 split of the coefficient block, per j in group
        cfj = coef[:].rearrange("p (j r) -> p j r", j=JG)
        for jj in range(JG):
            j = g * JG + jj
            if osb is None:
                chunk = next(e - s for (s, e) in sched if s <= j < e)
                osb = osbp.tile([128, chunk, OUTW], F32, tag="osb")
                osb_base = j
            tp = pst.tile([128, 128], F32)
            nc.tensor.transpose(tp[:], cfj[:, jj, :], i_sb[:])
            if f16x3:
                chi = lhsp.tile([128, 128], F16, tag="chi")
                clo = lhsp.tile([128, 128], F16, tag="clo")
                nc.scalar.copy(chi[:], tp[:])
                nc.vector.tensor_sub(clo[:], tp[:], chi[:])
                lhs = (chi, clo)
            else:
                lh = lhsp.tile([128, 128], F32, tag="lh")
                nc.scalar.copy(lh[:], tp[:])
                lhs = (lh,)

            po0 = psop.tile([128, 1024], F32, tag="po")
            po1 = psop.tile([128, 1024], F32, tag="po")
            for c in range(4):
                dst = (po0 if c < 2 else po1)[:, (c % 2) * 512:(c % 2 + 1) * 512]
                if f16x3:
                    passes = [(lhs[0], wmats[0]), (lhs[0], wmats[1]),
                              (lhs[1], wmats[0])]
                else:
                    passes = [(lhs[0], wmats[0])]
                for i, (lt, wt) in enumerate(passes):
                    kw = dict(start=(i == 0), stop=(i == len(passes) - 1))
                    if MATMUL_MODE == "rowtile":
                        nc.tensor.matmul(
                            dst, lt[32 * c:32 * c + 32, :],
                            wt[32 * c:32 * c + 32, 512 * c:512 * (c + 1)],
                            tile_position=(32 * c, 0), **kw)
                    else:
                        nc.tensor.matmul(
                            dst, lt[:], wt[:, 512 * c:512 * (c + 1)], **kw)

            nv = DVE_COPY_COLS
            nc.vector.tensor_copy(osb[:, j - osb_base, 0:nv], po0[:, 0:nv])
            nc.scalar.copy(osb[:, j - osb_base, nv:1024], po0[:, nv:1024])
            nc.scalar.copy(osb[:, j - osb_base, 1024:2048], po1[:])
            if j in dma_of:
                a, b = dma_of[j]
                eng = nc.scalar if (DUAL_DMA and (a // OSG) % 2) else nc.sync
                eng.dma_start(outj[:, a:b, :], osb[:])
                osb = None


def _phase_b(nc, work, xv, eps_t, zero_t):
    """Knots + cubic coefficients for JG j's. Returns coef [128, JG*128]."""
    vec = nc.vector
    diff = work.tile([128, JG * 18 * 2], F32, name="diff")
    sq = work.tile([128, JG * 18 * 2], F32, name="sq")
    ss = work.tile([128, JG * 18], F32, name="ss")
    sqt = work.tile([128, JG * 18], F32, name="sqt")
    sl = work.tile([128, JG * 18], F32, name="sl")
    rsl = work.tile([128, JG * 18], F32, name="rsl")
    coef = work.tile([128, JG * 128], F32, name="coef")

    dv = diff[:].rearrange("p (j i d) -> p j i d", j=JG, i=18, d=D)
    qv = sq[:].rearrange("p (j i d) -> p j i d", j=JG, i=18, d=D)
    ssv = ss[:].rearrange("p (j i) -> p j i", j=JG)
    slv = sl[:].rearrange("p (j i) -> p j i", j=JG)
    rslv = rsl[:].rearrange("p (j i) -> p j i", j=JG)
    cfv = coef[:].rearrange("p (j s d k) -> p j s d k", j=JG, s=S, d=D, k=4)

    # inner aux diffs: D[i] = cps[i] - cps[i-1] (i=1..15), D[16] = cps0 - cps15
    vec.tensor_sub(dv[:, :, 1:16, :], xv[:, :, 1:16, :], xv[:, :, 0:15, :])
    vec.tensor_sub(dv[:, :, 16, :], xv[:, :, 0, :], xv[:, :, 15, :])
    vec.tensor_mul(qv[:, :, 1:17, :], dv[:, :, 1:17, :], dv[:, :, 1:17, :])
    vec.tensor_add(ssv[:, :, 1:17], qv[:, :, 1:17, 0], qv[:, :, 1:17, 1])

    l01 = work.tile([128, JG], F32, name="l01")
    llast = work.tile([128, JG], F32, name="llast")
    rl01 = work.tile([128, JG], F32, name="rl01")
    rllast = work.tile([128, JG], F32, name="rllast")
    r1 = work.tile([128, JG], F32, name="r1")
    r2 = work.tile([128, JG], F32, name="r2")
    # l01/llast include the +EPS inside the sqrt (as the reference does)
    nc.scalar.activation(l01[:], ssv[:, :, 1], AF.Sqrt, bias=eps_t[:])
    nc.scalar.activation(llast[:], ssv[:, :, 16], AF.Sqrt, bias=eps_t[:])
    vec.reciprocal(rl01[:], l01[:])
    vec.reciprocal(rllast[:], llast[:])
    vec.tensor_mul(r1[:], l01[:], rllast[:])     # l01/llast
    vec.tensor_mul(r2[:], llast[:], rl01[:])     # llast/l01
    for d in range(D):
        vec.tensor_mul(dv[:, :, 0, d], r1[:], dv[:, :, 16, d])
        vec.tensor_mul(dv[:, :, 17, d], r2[:], dv[:, :, 1, d])
    vec.tensor_mul(qv[:, :, 0, :], dv[:, :, 0, :], dv[:, :, 0, :])
    vec.tensor_mul(qv[:, :, 17, :], dv[:, :, 17, :], dv[:, :, 17, :])
    vec.tensor_add(ssv[:, :, 0], qv[:, :, 0, 0], qv[:, :, 0, 1])
    vec.tensor_add(ssv[:, :, 17], qv[:, :, 17, 0], qv[:, :, 17, 1])

    # seg_len = ss^(1/4); knot diffs are sums of consecutive seg_lens
    nc.scalar.activation(sqt[:], ss[:], AF.Sqrt, bias=zero_t[:])
    nc.scalar.activation(sl[:], sqt[:], AF.Sqrt, bias=zero_t[:])
    vec.reciprocal(rsl[:], sl[:])

    d20 = work.tile([128, JG * S], F32, name="d20")
    d31 = work.tile([128, JG * S], F32, name="d31")
    r20 = work.tile([128, JG * S], F32, name="r20")
    r31 = work.tile([128, JG * S], F32, name="r31")
    b01 = work.tile([128, JG * S], F32, name="b01")
    b23 = work.tile([128, JG * S], F32, name="b23")
    p20 = work.tile([128, JG * S], F32, name="p20")
    q20 = work.tile([128, JG * S], F32, name="q20")
    q31 = work.tile([128, JG * S], F32, name="q31")

    def segv(t):
        return t[:].rearrange("p (j s) -> p j s", j=JG)

    vec.tensor_add(segv(d20), slv[:, :, 0:16], slv[:, :, 1:17])
    vec.tensor_add(segv(d31), slv[:, :, 1:17], slv[:, :, 2:18])
    vec.reciprocal(r20[:], d20[:])
    vec.reciprocal(r31[:], d31[:])
    vec.tensor_mul(segv(b01), slv[:, :, 1:17], rslv[:, :, 0:16])
    vec.tensor_mul(segv(b23), slv[:, :, 1:17], rslv[:, :, 2:18])
    vec.tensor_mul(segv(p20), slv[:, :, 0:16], segv(r20))
    vec.tensor_mul(segv(q20), slv[:, :, 1:17], segv(r20))
    vec.tensor_mul(segv(q31), slv[:, :, 1:17], segv(r31))

    # assembly with the per-segment scalar arrays broadcast over d (step-0 AP)
    def bc(t):
        ap = segv(t)
        return bass.AP(ap.tensor, ap.offset, ap.ap + [[0, 2]])

    sc = [work.tile([128, JG * S * D], F32, name=f"sc{i}") for i in range(6)]
    b01d, b23d, dbd, ead, md, c2d = (
        t[:].rearrange("p (j s d) -> p j s d", j=JG, s=S) for t in sc)
    dd0 = dv[:, :, 0:16, :]    # D[s]
    dd1 = dv[:, :, 1:17, :]    # D[s+1] = B12
    dd2 = dv[:, :, 2:18, :]    # D[s+2]
    g0 = cfv[:, :, :, :, 0]
    g1 = cfv[:, :, :, :, 1]
    g2 = cfv[:, :, :, :, 2]
    g3 = cfv[:, :, :, :, 3]
    vec.tensor_mul(b01d, bc(b01), dd0)
    vec.tensor_mul(b23d, bc(b23), dd2)
    vec.tensor_sub(dbd, dd1, b01d)          # dB = B12 - B01
    vec.tensor_sub(ead, b23d, dd1)          # eA' = B23 - B12
    vec.tensor_mul(md, bc(q31), ead)        # h2
    vec.tensor_mul(c2d, bc(q20), dbd)       # c2
    vec.tensor_sub(g3, md, c2d)             # g3 = h2 - c2
    vec.tensor_mul(md, bc(p20), dbd)        # reuse md as t1
    vec.tensor_add(g1, b01d, md)            # g1 = B01 + p20*dB
    vec.tensor_sub(c2d, dd1, g1)            # reuse c2d as B12 - g1
    vec.tensor_sub(g2, c2d, g3)             # g2 = B12 - g1 - g3
    vec.tensor_copy(g0, xv)                 # g0 = P1 = cps[s]
    return coef


_NC_CACHE = {}


def _get_nc(n_reps: int = 1, loop_n: int = 0):
    key = (n_reps, loop_n, MATMUL_MODE, MM_DTYPE, JG, OSG, DUAL_DMA,
           DVE_COPY_COLS, DMA_SCHED, INPUT_SPLIT, WARM_MM)
    if key not in _NC_CACHE:
        _NC_CACHE[key] = _build_nc(n_reps, loop_n)
    return _NC_CACHE[key]


def _input_arrays():
    w = _build_w()
    arrs = {"ident": np.eye(128, dtype=np.float32)}
    if MM_DTYPE == "f16x3":
        whi = w.astype(np.float16)
        wlo = (w - whi.astype(np.float32)).astype(np.float16)
        arrs["whi"] = whi
        arrs["wlo"] = wlo
    else:
        arrs["wmat"] = w
    return arrs


def run(cps: np.ndarray, trace: bool = False, trace_cores=None):
    cps = np.ascontiguousarray(np.asarray(cps, dtype=np.float32))
    assert cps.shape == (B_FULL, S, D), cps.shape
    nc = _get_nc()
    arrs = _input_arrays()
    flat = cps.reshape(N_CORES, BC, S * D)
    in_maps = [dict(arrs, cps=flat[c]) for c in range(N_CORES)]
    res = run_bass_kernel_spmd(
        nc, in_maps, list(range(N_CORES)), trace=trace,
        trace_cores=trace_cores,
    )
    out = np.concatenate([res.results[c]["out"] for c in range(N_CORES)], axis=0)
    return out.reshape(B_FULL, S * P, D), res


def kernel(cps: np.ndarray) -> np.ndarray:
    out, _ = run(cps, trace=False)
    return out

